# revision 2
# baseline (speedup 1.0000x reference)
"""Trainium2 Bass kernel for nn_patch_expanding.

Computes, for x [32, 1024, 1024] and w [512, 512]:
    xg = x.reshape(B, 32, 32, 1024); x0, x1 = split(xg, channel halves)
    xi = row-interleave(x0, x1) -> [B, 64, 32, 512]
    y  = xi @ w -> reshape [B, 2048, 512]

Data-parallel over batch (4 batches/core on 8 cores); fp16 on device
(host rounds inputs; rel err ~5e-4 vs the 2e-2 gate). Per core per rep:
x [4096 tok, 1024 ch] -> y [8192, 512].

v7: w-stationary yT formulation.
- The per-core job is y[8192,512] = xi[8192,512] @ w, where xi is a row
  permutation of x's two channel halves. Instead of making the x data the
  PE-stationary operand (v6: one LDWEIGHTS per matmul, M=32 col tiling),
  compute yT[cout, tok] = w[cin,cout].T-chunk @ xT[cin, tok]: the 16
  [128,128] w tiles are stationary (1 LDWEIGHTS per 8 matmuls) and x
  streams as the moving operand in plain token order. 256 N=512 fp16
  matmuls/rep -> ~55us PE roofline. The channel-half interleave becomes a
  pure output-indexing problem handled on the host during unshard.
- x enters cin-major via HBM->SBUF DMA XBAR transposes [2048,128]->[128,2048].
  HWDGE descriptor-gen costs ~34ns/16x128-tile = ~4.4us per transpose, so
  the 16 per rep are split across BOTH HWDGE rings (SP: ch-chunks 0-3,
  ACT: 4-7) -> ~35us/ring, under the PE. xt is quad-buffered.
- PE loop per (lg, c)-group: k-outer accumulation, 8 PSUM banks =
  (half s, token tile t); bank completions stagger so DVE evictions chase
  the k=3 pass and banks recycle without stalling the PE.
- DVE evicts PSUM->fp16 ysb (drain-fenced); stores of 1MB contiguous
  yT-tile groups are issued on the gpsimd SWDGE ring so the HWDGE rings
  carry nothing but transposes. Host reassembles y from the tile layout.
"""
import sys
sys.path.insert(0, "/opt/trn_rl_repo")
import numpy as np

B, L, C = 32, 1024, 1024
NCORES = 8
BPC = B // NCORES
ROWS = BPC * L             # 4096 tokens per core
TL = 2048                  # tokens per load-group
NGL = ROWS // TL           # 2 load-groups per rep
GPL = 4                    # (c) groups per load-group
GPR = NGL * GPL            # 8 psum-groups per rep (each: 8 banks of [128,512])

_CACHE = {}


def _build(reps: int = 1, sim: bool = False):
    import concourse.bass as bass
    from concourse import mybir

    f16, f32 = mybir.dt.float16, mybir.dt.float32
    nc = bass.Bass(trn_type="TRN2", target_bir_lowering=False, debug=False,
                   num_devices=NCORES)

    xd = nc.dram_tensor("x", [ROWS, C], f16, kind="ExternalInput").ap()
    wd = nc.dram_tensor("w", [512, 512], f16, kind="ExternalInput").ap()
    # yT tile layout: [group gg%8][cout part 128][bank slot 8][tok 512]
    yd = nc.dram_tensor("y", [GPR, 128, 8, 512], f16, kind="ExternalOutput").ap()

    s_lw = nc.alloc_semaphore("s_lw")
    s_tr = [nc.alloc_semaphore(f"s_tr{i}") for i in range(4)]
    s_mm = nc.alloc_semaphore("s_mm")    # +1 per completed psum bank
    s_ye = nc.alloc_semaphore("s_ye")    # +1 per DVE eviction
    s_yd = nc.alloc_semaphore("s_yd")    # +1 per drained evict-group
    s_st = nc.alloc_semaphore("s_st")    # +16 per completed store
    all_sems = s_tr + [s_lw, s_mm, s_ye, s_yd, s_st]

    GL = NGL * reps            # total load-groups
    G = GPR * reps             # total psum-groups
    W = 8 * G                  # total psum-bank completions

    def transposes(eng, lg, kks):
        par, la = lg % 4, lg % NGL
        if lg >= 4:
            # xt[par] free once PE consumed load-group lg-4
            eng.wait_ge(s_mm, 8 * GPL * (lg - 3))
        for kk in kks:
            eng.dma_start(
                xt_a[:, par, kk, :],
                xd[TL * la:TL * la + TL, 128 * kk:128 * kk + 128],
                transpose=True,
            ).then_inc(s_tr[par], 16)

    with (
        nc.sbuf_tensor("xt", [128, 4, 8, TL], f16) as xt,
        nc.sbuf_tensor("wsb", [128, 4, 4, 128], f16) as wsb,
        nc.sbuf_tensor("ysb", [128, 2, 8, 512], f16) as ysb,
        nc.psum_tensor("ps", [128, 8, 512], f32) as ps,
    ):
        xt_a, wsb_a, ysb_a, ps_a = xt.ap(), wsb.ap(), ysb.ap(), ps.ap()

        if not sim:
            for s in all_sems:
                nc.gpsimd.sem_clear(s)
            for eng in (nc.sync, nc.tensor, nc.vector, nc.scalar):
                for _ in range(4):
                    eng.nop(cycle_cnt=6000, nofuse=True)

        with nc.Block() as block:

            @block.gpsimd
            def _(g):
                # stores: one 1MB contiguous store per completed psum-group
                for gg in range(G):
                    g.wait_ge(s_yd, gg + 1)
                    g.dma_start(
                        yd[gg % GPR], ysb_a[:, gg % 2, :, :],
                    ).then_inc(s_st, 16)
                g.wait_ge(s_st, 16 * G)
                if not sim:
                    for s in all_sems:
                        g.sem_clear(s)

            @block.sync
            def _(sp):
                sp.dma_start(
                    wsb_a[:],
                    wd.rearrange("(k p) (c m) -> p k c m", p=128, m=128),
                ).then_inc(s_lw, 16)
                for lg in range(GL):
                    transposes(sp, lg, range(4))

            @block.scalar
            def _(ac):
                for lg in range(GL):
                    transposes(ac, lg, range(4, 8))

            @block.tensor
            def _(pe):
                pe.wait_ge(s_lw, 16)
                for gg in range(G):
                    lg, c = gg // GPL, gg % GPL
                    par = lg % 4
                    if c == 0:
                        pe.wait_ge(s_tr[par], 128 * (lg // 4 + 1))
                    for k in range(4):
                        for b in range(8):          # bank = 4*s + t
                            s, t = b // 4, b % 4
                            if k == 0 and gg >= 1:
                                pe.wait_ge(s_ye, 8 * (gg - 1) + b + 1)
                            inst = pe.matmul(
                                ps_a[:, b, :],
                                wsb_a[:, k, c, :],
                                xt_a[:, par, 4 * s + k, 512 * t:512 * t + 512],
                                start=(k == 0), stop=(k == 3),
                            )
                            if k == 3:
                                inst.then_inc(s_mm)

            @block.vector
            def _(dv):
                for gg in range(G):
                    parS = gg % 2
                    for b in range(8):
                        if b == 0 and gg >= 2:
                            dv.wait_ge(s_st, 16 * (gg - 1))  # ysb[parS] free
                        dv.wait_ge(s_mm, 8 * gg + b + 1)
                        dv.tensor_copy(ysb_a[:, parS, b, :], ps_a[:, b, :]
                                       ).then_inc(s_ye)
                    # visibility barrier for the SWDGE store of this group
                    dv.drain().then_inc(s_yd)

    return nc


def _in_maps(x: np.ndarray, w: np.ndarray) -> list:
    xs = np.ascontiguousarray(x, dtype=np.float16).reshape(NCORES, ROWS, C)
    wh = np.ascontiguousarray(w, dtype=np.float16)
    return [{"x": xs[i], "w": wh} for i in range(NCORES)]


def _unshard(yts: np.ndarray) -> np.ndarray:
    """yts [NCORES, GPR, 128, 8, 512] fp16 -> y [B, 2L, 512] fp32.

    Device tile (gg=(lg,c), p, slot=(s,t), u) holds
    yT[cout=128c+p, x_tok=2048lg+512t+u] for channel half s, where
    x_tok = 1024*b'' + 32h + w  (b'' in 0..3 per core) and the output row
    is 64h + 32s + w.  Decompose t=(th,tp): b''=2lg+th; u=(h2,wc):
    h=16*tp+h2 -> row = 1024*tp + 64*h2 + 32*s + wc.
    """
    a = yts.reshape(NCORES, 2, 4, 128, 2, 2, 2, 16, 32)
    #                core, lg,  c,   p, s, th, tp, h2, wc
    a = a.transpose(0, 1, 5, 6, 7, 4, 8, 2, 3)
    #   core, lg, th, tp, h2, s, wc, c, p
    return a.reshape(B, 2 * L, C // 2).astype(np.float32)


def kernel(x: np.ndarray, w: np.ndarray) -> np.ndarray:
    from concourse.bass_utils import run_bass_kernel_spmd

    if "nc" not in _CACHE:
        _CACHE["nc"] = _build()
    nc = _CACHE["nc"]

    in_maps = _in_maps(x, w)
    res = run_bass_kernel_spmd(nc, in_maps, list(range(NCORES)))
    yts = np.stack([res.results[i]["y"] for i in range(NCORES)], axis=0)
    return _unshard(yts)
